# revision 18
# baseline (speedup 1.0000x reference)
"""ExiGCN layer (nn_ExiGCNLayer) on 8 Trainium2 NeuronCores.

Strategy (1D graph partitioning, per the sharding hint):
 - Nodes are packed into 400 tiles of 125 (4-way degree-balanced greedy
   packing), 50 tiles per core; each core owns its tiles' output rows.
 - The feature table cat(H, dH) [N, 256] (shipped fp16) is replicated to
   every core; edge lists are bucketed by destination tile and padded to a
   uniform chunk count so all 8 cores run one identical SPMD program.
 - Per 128-edge chunk: dma_gather pulls 128 rows (512 B fp16 each) of the
   table into SBUF, one row per partition. A selection matrix
   MT[e, j] = w[e] * (dloc[e] == j), built with one fused DVE tensor_scalar
   (is_equal x mult) against an iota row, turns the scaled segment-sum into
   ONE 128x128x256 PE matmul per chunk (MT stationary, G moving):
   acc[dloc, 0:256] += MT.T @ G for adj chunks, acc[dloc, 256:512] for
   delta-adj chunks, all accumulating in a single PSUM bank group.
 - Per tile: F = AdH + dAH + dAdH and B = AH + F via DVE adds; F and B are
   PE-transposed to serve as stationary operands (fp32r, the fast-fp32 PE
   mode) for fixed = F @ W and Z' = cached_Z + db + F @ W + B @ dW.
   new_Z | fixed | B are staged in one SBUF tile and stored with one DMA.
 - dma_gather requires int16 indices, so each tile's edges are split into
   src < 32768 and src >= 32768 groups gathered from the two table halves
   (single_packet=False -- large single-packet gathers hang the device).

Numerics: fp16 gathered features/weights, fp32 PSUM accumulation, fp32r
epilogue matmuls; max relative error vs the fp32 reference ~3.5e-4.

kernel(**inputs) -> (new_Z, fixed_term, B), all [50000, 128] float32.
"""
import sys

if "/opt/trn_rl_repo" not in sys.path:
    sys.path.insert(0, "/opt/trn_rl_repo")

import numpy as np

import concourse.bass as bass
import concourse.mybir as mybir
import concourse.tile as tile
from concourse import bacc
from concourse.bass_utils import run_bass_kernel_spmd
from concourse.masks import make_identity

N = 50000
D = 128
OUT = 128
P = 128
NODES_PER_TILE = 125
T_TILES = 400
NCORES = 8
TILES_PER_CORE = T_TILES // NCORES
SPLIT = 32768
ROWS_PER_CORE = TILES_PER_CORE * NODES_PER_TILE  # 6250

FP = mybir.dt.float32
F32R = mybir.dt.float32r
F16 = mybir.dt.float16
I16 = mybir.dt.int16
I32 = mybir.dt.int32


# ----------------------------------------------------------------------------
# host-side prep
# ----------------------------------------------------------------------------

def _pack_nodes(deg4):
    """Greedy 4-way balanced packing of nodes into tiles (then snake tail)."""
    total = deg4.sum(axis=1)
    order = np.argsort(-total, kind="stable")
    target = np.maximum(deg4.sum(axis=0).astype(np.float64) / T_TILES, 1.0)
    loads = np.zeros((T_TILES, 4), dtype=np.float64)
    counts = np.zeros(T_TILES, dtype=np.int64)
    tile_of = np.empty(N, dtype=np.int64)
    slot_of = np.empty(N, dtype=np.int64)
    head = min(N, 8 * T_TILES)
    inv_t = 1.0 / target
    for i in range(head):
        v = order[i]
        cand = (loads + deg4[v]) * inv_t
        score = cand.max(axis=1)
        score[counts >= NODES_PER_TILE] = np.inf
        t = int(np.argmin(score))
        tile_of[v] = t
        slot_of[v] = counts[t]
        counts[t] += 1
        loads[t] += deg4[v]
    tpos, direction = 0, 1
    for v in order[head:]:
        for _ in range(2 * T_TILES):
            if counts[tpos] < NODES_PER_TILE:
                break
            tpos += direction
            if tpos >= T_TILES:
                tpos, direction = T_TILES - 1, -1
            elif tpos < 0:
                tpos, direction = 0, 1
        t = tpos
        tile_of[v] = t
        slot_of[v] = counts[t]
        counts[t] += 1
        loads[t] += deg4[v]
        tpos += direction
        if tpos >= T_TILES:
            tpos, direction = T_TILES - 1, -1
        elif tpos < 0:
            tpos, direction = 0, 1
    assert (counts == NODES_PER_TILE).all()
    return tile_of, slot_of


def _prep(inputs):
    adj_src = np.asarray(inputs["adj_src"]).astype(np.int64)
    adj_dst = np.asarray(inputs["adj_dst"]).astype(np.int64)
    adj_w = np.asarray(inputs["adj_w"]).astype(np.float32)
    dadj_src = np.asarray(inputs["dadj_src"]).astype(np.int64)
    dadj_dst = np.asarray(inputs["dadj_dst"]).astype(np.int64)
    dadj_w = np.asarray(inputs["dadj_w"]).astype(np.float32)

    deg4 = np.zeros((N, 4), dtype=np.int64)
    np.add.at(deg4[:, 0], adj_dst[adj_src < SPLIT], 1)
    np.add.at(deg4[:, 1], adj_dst[adj_src >= SPLIT], 1)
    np.add.at(deg4[:, 2], dadj_dst[dadj_src < SPLIT], 1)
    np.add.at(deg4[:, 3], dadj_dst[dadj_src >= SPLIT], 1)

    tile_of, slot_of = _pack_nodes(deg4)

    def bucket(src, dst):
        key = tile_of[dst] * 2 + (src >= SPLIT)
        perm = np.argsort(key, kind="stable")
        cnt = np.bincount(key, minlength=T_TILES * 2).reshape(T_TILES, 2)
        return perm, cnt

    perm_a, cnt_a = bucket(adj_src, adj_dst)
    perm_d, cnt_d = bucket(dadj_src, dadj_dst)

    C_la = max(1, int(np.ceil(cnt_a[:, 0].max() / P)))
    C_ha = max(1, int(np.ceil(cnt_a[:, 1].max() / P)))
    C_ld = int(np.ceil(cnt_d[:, 0].max() / P))
    C_hd = int(np.ceil(cnt_d[:, 1].max() / P))
    C_LO = C_la + C_ld
    C_HI = C_ha + C_hd
    CH_T = C_LO + C_HI
    SLOTS_T = CH_T * P

    idx_rel = np.zeros(T_TILES * SLOTS_T, dtype=np.int16)
    w_arr = np.zeros(T_TILES * SLOTS_T, dtype=np.float32)
    dloc_arr = np.zeros(T_TILES * SLOTS_T, dtype=np.float32)

    def fill(perm, cnt, src, dst, w, col, region_off, cap, base_sub):
        sel = perm[(src[perm] >= SPLIT) == bool(col)]
        t_e = tile_of[dst[sel]]
        c = cnt[:, col]
        start = np.zeros(T_TILES, dtype=np.int64)
        np.cumsum(c[:-1], out=start[1:])
        ranks = np.arange(len(sel)) - start[t_e]
        assert (ranks >= 0).all() and (ranks < cap).all()
        slot = t_e * SLOTS_T + region_off + ranks
        idx_rel[slot] = (src[sel] - base_sub).astype(np.int16)
        w_arr[slot] = w[sel]
        dloc_arr[slot] = slot_of[dst[sel]].astype(np.float32)

    fill(perm_a, cnt_a, adj_src, adj_dst, adj_w, 0, 0, C_la * P, 0)
    fill(perm_d, cnt_d, dadj_src, dadj_dst, dadj_w, 0, C_la * P, C_ld * P, 0)
    fill(perm_a, cnt_a, adj_src, adj_dst, adj_w, 1, C_LO * P, C_ha * P, SPLIT)
    fill(perm_d, cnt_d, dadj_src, dadj_dst, dadj_w, 1,
         (C_LO + C_ha) * P, C_hd * P, SPLIT)

    # wd interleave: wd_dev[t][p, 2*ck] = dloc, [p, 2*ck+1] = w
    w3 = w_arr.reshape(T_TILES, CH_T, P)
    d3 = dloc_arr.reshape(T_TILES, CH_T, P)
    wd = np.empty((T_TILES, CH_T, 2, P), dtype=np.float32)
    wd[:, :, 0, :] = d3
    wd[:, :, 1, :] = w3
    wd_dev = wd.reshape(T_TILES, CH_T * 2, P).transpose(0, 2, 1)
    wd_cores = np.ascontiguousarray(
        wd_dev.reshape(NCORES, TILES_PER_CORE, P, 2 * CH_T).transpose(0, 2, 1, 3)
    ).reshape(NCORES, P, TILES_PER_CORE * 2 * CH_T)

    # idx int16 packing: flat slot i -> (partition i%16, col i//16), 8x replicated
    flat = idx_rel.reshape(T_TILES, SLOTS_T)
    cols_t = SLOTS_T // 16
    packed = flat.reshape(T_TILES, cols_t, 16).transpose(0, 2, 1)
    idx_dev = np.empty((T_TILES, P, cols_t), dtype=np.int16)
    for g in range(8):
        idx_dev[:, g * 16 : (g + 1) * 16, :] = packed
    idx_cores = np.ascontiguousarray(
        idx_dev.reshape(NCORES, TILES_PER_CORE, P, cols_t).transpose(0, 2, 1, 3)
    ).reshape(NCORES, P, TILES_PER_CORE * cols_t)

    node_at = np.empty(N, dtype=np.int64)
    node_at[tile_of * NODES_PER_TILE + slot_of] = np.arange(N)

    meta = (C_la, C_ha, C_ld, C_hd)
    return meta, idx_cores, wd_cores, node_at, tile_of, slot_of


# ----------------------------------------------------------------------------
# device program
# ----------------------------------------------------------------------------

_PROGRAM_CACHE = {}


def _build_program(meta, n_tiles=TILES_PER_CORE, variant="full", passes=1):
    key = (meta, n_tiles, variant, passes)
    if key in _PROGRAM_CACHE:
        return _PROGRAM_CACHE[key]
    C_la, C_ha, C_ld, C_hd = meta
    C_LO = C_la + C_ld
    C_HI = C_ha + C_hd
    CH_T = C_LO + C_HI
    cols_t = CH_T * P // 16  # int16 idx columns per tile

    nc = bacc.Bacc("TRN2", target_bir_lowering=False, debug=False,
                   num_devices=NCORES)
    table_d = nc.dram_tensor("table", [N, 256], F16, kind="ExternalInput").ap()
    idx_d = nc.dram_tensor("idxs", [P, TILES_PER_CORE * cols_t], I16,
                           kind="ExternalInput").ap()
    wd_d = nc.dram_tensor("wd", [P, TILES_PER_CORE * 2 * CH_T], FP,
                          kind="ExternalInput").ap()
    zb_d = nc.dram_tensor("zb", [ROWS_PER_CORE, OUT], FP,
                          kind="ExternalInput").ap()
    W_d = nc.dram_tensor("Wm", [D, OUT], F32R, kind="ExternalInput").ap()
    dW_d = nc.dram_tensor("dWm", [D, OUT], F32R, kind="ExternalInput").ap()
    # res columns: [0:128] new_Z, [128:256] fixed_term, [256:384] B
    res_d = nc.dram_tensor("res", [ROWS_PER_CORE, 3 * OUT], FP,
                           kind="ExternalOutput").ap()

    with tile.TileContext(nc) as tc:
        with (
            tc.tile_pool(name="const", bufs=1) as cpool,
            tc.tile_pool(name="gp", bufs=4) as gpool,
            tc.tile_pool(name="zbp", bufs=4) as zbpool,
            tc.tile_pool(name="mt", bufs=8) as mtpool,
            tc.tile_pool(name="ep", bufs=3) as epool,
            tc.tile_pool(name="pacc", bufs=2, space="PSUM") as pacc,
            tc.tile_pool(name="ptr", bufs=2, space="PSUM") as ptr,
            tc.tile_pool(name="pout", bufs=2, space="PSUM") as pout,
        ):
            W_sb = cpool.tile([D, OUT], F32R)
            dW_sb = cpool.tile([D, OUT], F32R)
            nc.sync.dma_start(W_sb[:], W_d[:])
            nc.sync.dma_start(dW_sb[:], dW_d[:])
            iota_i = cpool.tile([P, P], I32)
            nc.gpsimd.iota(iota_i[:], pattern=[[1, P]], base=0,
                           channel_multiplier=0)
            iota_f = cpool.tile([P, P], F16)
            nc.vector.tensor_copy(iota_f[:], iota_i[:])
            ident = cpool.tile([P, P], FP)
            make_identity(nc, ident[:])
            idx_all = cpool.tile([P, n_tiles * cols_t], I16)
            nc.sync.dma_start(idx_all[:], idx_d[:, : n_tiles * cols_t])
            wd_all = cpool.tile([P, n_tiles * 2 * CH_T], FP)
            nc.sync.dma_start(wd_all[:], wd_d[:, : n_tiles * 2 * CH_T])

            for t_iter in range(n_tiles * passes):
                t = t_iter % n_tiles
                zb_t = zbpool.tile([P, OUT], FP, tag="zb")
                nc.sync.dma_start(
                    zb_t[:NODES_PER_TILE, :],
                    zb_d[t * NODES_PER_TILE : (t + 1) * NODES_PER_TILE, :])

                G = gpool.tile([P, CH_T * 256], F16, tag="g")
                c0 = t * cols_t
                nc.gpsimd.dma_gather(
                    out_ap=G[:, : C_LO * 256].rearrange(
                        "p (c f) -> p c f", c=C_LO),
                    in_ap=table_d[:SPLIT, :],
                    idxs_ap=idx_all[:, c0 : c0 + C_LO * 8],
                    num_idxs=C_LO * P,
                    num_idxs_reg=C_LO * P,
                    elem_size=256,
                    single_packet=False,
                )
                nc.gpsimd.dma_gather(
                    out_ap=G[:, C_LO * 256 :].rearrange(
                        "p (c f) -> p c f", c=C_HI),
                    in_ap=table_d[SPLIT:, :],
                    idxs_ap=idx_all[:, c0 + C_LO * 8 : c0 + cols_t],
                    num_idxs=C_HI * P,
                    num_idxs_reg=C_HI * P,
                    elem_size=256,
                    single_packet=False,
                )

                # acc[dloc, 0:128]=AH  [128:256]=AdH  [256:384]=dAH  [384:512]=dAdH
                acc = pacc.tile([P, 512], FP, tag="acc")
                w0 = t * 2 * CH_T
                for ck in range(CH_T):
                    MT = mtpool.tile([P, P], F16, tag="mt")
                    nc.vector.tensor_scalar(
                        out=MT[:],
                        in0=iota_f[:],
                        scalar1=wd_all[:, w0 + 2 * ck : w0 + 2 * ck + 1],
                        scalar2=wd_all[:, w0 + 2 * ck + 1 : w0 + 2 * ck + 2],
                        op0=mybir.AluOpType.is_equal,
                        op1=mybir.AluOpType.mult,
                    )
                    dadj = (C_la <= ck < C_LO) or (ck >= C_LO + C_ha)
                    base = 256 if dadj else 0
                    nc.tensor.matmul(
                        acc[:, base : base + 256],
                        lhsT=MT[:],
                        rhs=G[:, ck * 256 : (ck + 1) * 256],
                        start=(ck == 0),
                        stop=(ck == CH_T - 1),
                    )

                # F = AdH + dAH + dAdH ; B = AH + F
                F_sb = epool.tile([P, P], FP, tag="ft")
                nc.vector.tensor_copy(F_sb[:], acc[:, 128:256])
                nc.vector.tensor_tensor(
                    out=F_sb[:], in0=F_sb[:], in1=acc[:, 256:384],
                    op=mybir.AluOpType.add)
                nc.vector.tensor_tensor(
                    out=F_sb[:], in0=F_sb[:], in1=acc[:, 384:512],
                    op=mybir.AluOpType.add)
                stage = epool.tile([P, 3 * OUT], FP, tag="stage")
                nc.vector.tensor_tensor(
                    out=stage[:, 256:384], in0=F_sb[:], in1=acc[:, 0:128],
                    op=mybir.AluOpType.add)

                ft_ps = ptr.tile([P, P], FP, tag="ftp", space="PSUM")
                nc.tensor.transpose(ft_ps[:], F_sb[:], ident[:])
                bt_ps = ptr.tile([P, P], FP, tag="btp", space="PSUM")
                nc.tensor.transpose(bt_ps[:], stage[:, 256:384], ident[:])
                FT_sb = epool.tile([P, P], F32R, tag="ftt")
                nc.scalar.copy(FT_sb[:], ft_ps[:])
                BT_sb = epool.tile([P, P], F32R, tag="btt")
                nc.scalar.copy(BT_sb[:], bt_ps[:])

                pz = pout.tile([P, 2 * OUT], FP, tag="pz", space="PSUM")
                nc.tensor.matmul(pz[:, 128:256], lhsT=FT_sb[:], rhs=W_sb[:],
                                 start=True, stop=False)
                nc.tensor.matmul(pz[:, 0:128], lhsT=FT_sb[:], rhs=W_sb[:],
                                 start=False, stop=False)
                nc.tensor.matmul(pz[:, 0:128], lhsT=BT_sb[:], rhs=dW_sb[:],
                                 start=False, stop=True)

                nc.scalar.copy(stage[:, 128:256], pz[:, 128:256])
                nc.vector.tensor_tensor(
                    out=stage[:NODES_PER_TILE, 0:128],
                    in0=pz[:NODES_PER_TILE, 0:128],
                    in1=zb_t[:NODES_PER_TILE, :],
                    op=mybir.AluOpType.add)

                r0 = t * NODES_PER_TILE
                nc.scalar.dma_start(
                    res_d[r0 : r0 + NODES_PER_TILE, :],
                    stage[:NODES_PER_TILE, :])

    nc.compile()
    _PROGRAM_CACHE[key] = nc
    return nc


# ----------------------------------------------------------------------------
# entry point
# ----------------------------------------------------------------------------

def _make_in_maps(inputs, meta, idx_cores, wd_cores, node_at):
    features = np.asarray(inputs["features"], dtype=np.float32)
    delta_features = np.asarray(inputs["delta_features"], dtype=np.float32)
    cached_Z = np.asarray(inputs["cached_Z"], dtype=np.float32)
    W = np.ascontiguousarray(np.asarray(inputs["W"], dtype=np.float32))
    dW = np.ascontiguousarray(np.asarray(inputs["delta_W"], dtype=np.float32))
    db = np.asarray(inputs["delta_bias"], dtype=np.float32)

    table = np.ascontiguousarray(
        np.concatenate([features, delta_features], axis=1).astype(np.float16))
    zb_perm = (cached_Z + db[None, :])[node_at].astype(np.float32)
    zb_cores = np.ascontiguousarray(
        zb_perm.reshape(NCORES, ROWS_PER_CORE, OUT))

    in_maps = []
    for c in range(NCORES):
        in_maps.append({
            "table": table,
            "idxs": np.ascontiguousarray(idx_cores[c]),
            "wd": np.ascontiguousarray(wd_cores[c]),
            "zb": zb_cores[c],
            "Wm": W,
            "dWm": dW,
        })
    return in_maps


def _assemble(results, node_at):
    res_rows = np.concatenate([r["res"] for r in results], axis=0)  # [N, 384]
    new_Z = np.empty((N, OUT), dtype=np.float32)
    fixed = np.empty((N, OUT), dtype=np.float32)
    B = np.empty((N, OUT), dtype=np.float32)
    new_Z[node_at] = res_rows[:, 0:128]
    fixed[node_at] = res_rows[:, 128:256]
    B[node_at] = res_rows[:, 256:384]
    return new_Z, fixed, B


def kernel(**inputs):
    meta, idx_cores, wd_cores, node_at, _, _ = _prep(inputs)
    nc = _build_program(meta)
    in_maps = _make_in_maps(inputs, meta, idx_cores, wd_cores, node_at)
    res = run_bass_kernel_spmd(nc, in_maps, list(range(NCORES)), trace=False)
    return _assemble(res.results, node_at)


# revision 21
# speedup vs baseline: 1.0021x; 1.0021x over previous
"""ExiGCN layer (nn_ExiGCNLayer) on 8 Trainium2 NeuronCores.

Strategy (1D graph partitioning, per the sharding hint):
 - Nodes are packed into 400 tiles of 125 (4-way degree-balanced greedy
   packing), 50 tiles per core; each core owns its tiles' output rows.
 - The feature table cat(H, dH) [N, 256] (shipped fp16) is replicated to
   every core; edge lists are bucketed by destination tile and padded to a
   uniform chunk count so all 8 cores run one identical SPMD program.
 - Per 128-edge chunk: dma_gather pulls 128 rows (512 B fp16 each) of the
   table into SBUF, one row per partition. A selection matrix
   MT[e, j] = w[e] * (dloc[e] == j), built with one fused DVE tensor_scalar
   (is_equal x mult) against an iota row, turns the scaled segment-sum into
   ONE 128x128x256 PE matmul per chunk (MT stationary, G moving):
   acc[dloc, 0:256] += MT.T @ G for adj chunks, acc[dloc, 256:512] for
   delta-adj chunks, all accumulating in a single PSUM bank group.
 - Per tile: F = AdH + dAH + dAdH and B = AH + F via DVE adds; F and B are
   PE-transposed to serve as stationary operands (fp32r, the fast-fp32 PE
   mode) for fixed = F @ W and Z' = cached_Z + db + F @ W + B @ dW.
   new_Z | fixed | B are staged in one SBUF tile and stored with one DMA.
 - dma_gather requires int16 indices, so each tile's edges are split into
   src < 32768 and src >= 32768 groups gathered from the two table halves
   (single_packet=False -- large single-packet gathers hang the device).

Numerics: fp16 gathered features/weights, fp32 PSUM accumulation, fp32r
epilogue matmuls; max relative error vs the fp32 reference ~3.5e-4.

kernel(**inputs) -> (new_Z, fixed_term, B), all [50000, 128] float32.
"""
import sys

if "/opt/trn_rl_repo" not in sys.path:
    sys.path.insert(0, "/opt/trn_rl_repo")

import numpy as np

import concourse.bass as bass
import concourse.mybir as mybir
import concourse.tile as tile
from concourse import bacc
from concourse.bass_utils import run_bass_kernel_spmd
from concourse.masks import make_identity

N = 50000
D = 128
OUT = 128
P = 128
NODES_PER_TILE = 125
T_TILES = 400
NCORES = 8
TILES_PER_CORE = T_TILES // NCORES
SPLIT = 32768
ROWS_PER_CORE = TILES_PER_CORE * NODES_PER_TILE  # 6250

FP = mybir.dt.float32
F32R = mybir.dt.float32r
F16 = mybir.dt.float16
I16 = mybir.dt.int16
I32 = mybir.dt.int32


# ----------------------------------------------------------------------------
# host-side prep
# ----------------------------------------------------------------------------

def _pack_nodes(deg4):
    """Greedy 4-way balanced packing of nodes into tiles (then snake tail)."""
    total = deg4.sum(axis=1)
    order = np.argsort(-total, kind="stable")
    target = np.maximum(deg4.sum(axis=0).astype(np.float64) / T_TILES, 1.0)
    loads = np.zeros((T_TILES, 4), dtype=np.float64)
    counts = np.zeros(T_TILES, dtype=np.int64)
    tile_of = np.empty(N, dtype=np.int64)
    slot_of = np.empty(N, dtype=np.int64)
    head = min(N, 8 * T_TILES)
    inv_t = 1.0 / target
    for i in range(head):
        v = order[i]
        cand = (loads + deg4[v]) * inv_t
        score = cand.max(axis=1)
        score[counts >= NODES_PER_TILE] = np.inf
        t = int(np.argmin(score))
        tile_of[v] = t
        slot_of[v] = counts[t]
        counts[t] += 1
        loads[t] += deg4[v]
    tpos, direction = 0, 1
    for v in order[head:]:
        for _ in range(2 * T_TILES):
            if counts[tpos] < NODES_PER_TILE:
                break
            tpos += direction
            if tpos >= T_TILES:
                tpos, direction = T_TILES - 1, -1
            elif tpos < 0:
                tpos, direction = 0, 1
        t = tpos
        tile_of[v] = t
        slot_of[v] = counts[t]
        counts[t] += 1
        loads[t] += deg4[v]
        tpos += direction
        if tpos >= T_TILES:
            tpos, direction = T_TILES - 1, -1
        elif tpos < 0:
            tpos, direction = 0, 1
    assert (counts == NODES_PER_TILE).all()
    return tile_of, slot_of


def _prep(inputs):
    adj_src = np.asarray(inputs["adj_src"]).astype(np.int64)
    adj_dst = np.asarray(inputs["adj_dst"]).astype(np.int64)
    adj_w = np.asarray(inputs["adj_w"]).astype(np.float32)
    dadj_src = np.asarray(inputs["dadj_src"]).astype(np.int64)
    dadj_dst = np.asarray(inputs["dadj_dst"]).astype(np.int64)
    dadj_w = np.asarray(inputs["dadj_w"]).astype(np.float32)

    deg4 = np.zeros((N, 4), dtype=np.int64)
    np.add.at(deg4[:, 0], adj_dst[adj_src < SPLIT], 1)
    np.add.at(deg4[:, 1], adj_dst[adj_src >= SPLIT], 1)
    np.add.at(deg4[:, 2], dadj_dst[dadj_src < SPLIT], 1)
    np.add.at(deg4[:, 3], dadj_dst[dadj_src >= SPLIT], 1)

    tile_of, slot_of = _pack_nodes(deg4)

    def bucket(src, dst):
        key = tile_of[dst] * 2 + (src >= SPLIT)
        perm = np.argsort(key, kind="stable")
        cnt = np.bincount(key, minlength=T_TILES * 2).reshape(T_TILES, 2)
        return perm, cnt

    perm_a, cnt_a = bucket(adj_src, adj_dst)
    perm_d, cnt_d = bucket(dadj_src, dadj_dst)

    C_la = max(1, int(np.ceil(cnt_a[:, 0].max() / P)))
    C_ha = max(1, int(np.ceil(cnt_a[:, 1].max() / P)))
    C_ld = int(np.ceil(cnt_d[:, 0].max() / P))
    C_hd = int(np.ceil(cnt_d[:, 1].max() / P))
    C_LO = C_la + C_ld
    C_HI = C_ha + C_hd
    CH_T = C_LO + C_HI
    SLOTS_T = CH_T * P

    idx_rel = np.zeros(T_TILES * SLOTS_T, dtype=np.int16)
    w_arr = np.zeros(T_TILES * SLOTS_T, dtype=np.float32)
    dloc_arr = np.zeros(T_TILES * SLOTS_T, dtype=np.float32)

    def fill(perm, cnt, src, dst, w, col, region_off, cap, base_sub):
        sel = perm[(src[perm] >= SPLIT) == bool(col)]
        t_e = tile_of[dst[sel]]
        c = cnt[:, col]
        start = np.zeros(T_TILES, dtype=np.int64)
        np.cumsum(c[:-1], out=start[1:])
        ranks = np.arange(len(sel)) - start[t_e]
        assert (ranks >= 0).all() and (ranks < cap).all()
        slot = t_e * SLOTS_T + region_off + ranks
        idx_rel[slot] = (src[sel] - base_sub).astype(np.int16)
        w_arr[slot] = w[sel]
        dloc_arr[slot] = slot_of[dst[sel]].astype(np.float32)

    fill(perm_a, cnt_a, adj_src, adj_dst, adj_w, 0, 0, C_la * P, 0)
    fill(perm_d, cnt_d, dadj_src, dadj_dst, dadj_w, 0, C_la * P, C_ld * P, 0)
    fill(perm_a, cnt_a, adj_src, adj_dst, adj_w, 1, C_LO * P, C_ha * P, SPLIT)
    fill(perm_d, cnt_d, dadj_src, dadj_dst, dadj_w, 1,
         (C_LO + C_ha) * P, C_hd * P, SPLIT)

    # wd interleave: wd_dev[t][p, 2*ck] = dloc, [p, 2*ck+1] = w
    w3 = w_arr.reshape(T_TILES, CH_T, P)
    d3 = dloc_arr.reshape(T_TILES, CH_T, P)
    wd = np.empty((T_TILES, CH_T, 2, P), dtype=np.float16)
    wd[:, :, 0, :] = d3
    wd[:, :, 1, :] = w3
    wd_dev = wd.reshape(T_TILES, CH_T * 2, P).transpose(0, 2, 1)
    wd_cores = np.ascontiguousarray(
        wd_dev.reshape(NCORES, TILES_PER_CORE, P, 2 * CH_T).transpose(0, 2, 1, 3)
    ).reshape(NCORES, P, TILES_PER_CORE * 2 * CH_T)

    # idx int16 packing: flat slot i -> (partition i%16, col i//16), 8x replicated
    flat = idx_rel.reshape(T_TILES, SLOTS_T)
    cols_t = SLOTS_T // 16
    packed = flat.reshape(T_TILES, cols_t, 16).transpose(0, 2, 1)
    idx_dev = np.empty((T_TILES, P, cols_t), dtype=np.int16)
    for g in range(8):
        idx_dev[:, g * 16 : (g + 1) * 16, :] = packed
    idx_cores = np.ascontiguousarray(
        idx_dev.reshape(NCORES, TILES_PER_CORE, P, cols_t).transpose(0, 2, 1, 3)
    ).reshape(NCORES, P, TILES_PER_CORE * cols_t)

    node_at = np.empty(N, dtype=np.int64)
    node_at[tile_of * NODES_PER_TILE + slot_of] = np.arange(N)

    meta = (C_la, C_ha, C_ld, C_hd)
    return meta, idx_cores, wd_cores, node_at, tile_of, slot_of


# ----------------------------------------------------------------------------
# device program
# ----------------------------------------------------------------------------

_PROGRAM_CACHE = {}


def _build_program(meta, n_tiles=TILES_PER_CORE, variant="full", passes=1):
    key = (meta, n_tiles, variant, passes)
    if key in _PROGRAM_CACHE:
        return _PROGRAM_CACHE[key]
    C_la, C_ha, C_ld, C_hd = meta
    C_LO = C_la + C_ld
    C_HI = C_ha + C_hd
    CH_T = C_LO + C_HI
    cols_t = CH_T * P // 16  # int16 idx columns per tile

    nc = bacc.Bacc("TRN2", target_bir_lowering=False, debug=False,
                   num_devices=NCORES)
    table_d = nc.dram_tensor("table", [N, 256], F16, kind="ExternalInput").ap()
    idx_d = nc.dram_tensor("idxs", [P, TILES_PER_CORE * cols_t], I16,
                           kind="ExternalInput").ap()
    wd_d = nc.dram_tensor("wd", [P, TILES_PER_CORE * 2 * CH_T], F16,
                          kind="ExternalInput").ap()
    zb_d = nc.dram_tensor("zb", [ROWS_PER_CORE, OUT], F16,
                          kind="ExternalInput").ap()
    W_d = nc.dram_tensor("Wm", [D, OUT], F32R, kind="ExternalInput").ap()
    dW_d = nc.dram_tensor("dWm", [D, OUT], F32R, kind="ExternalInput").ap()
    # res columns: [0:128] new_Z, [128:256] fixed_term, [256:384] B
    res_d = nc.dram_tensor("res", [ROWS_PER_CORE, 3 * OUT], F16,
                           kind="ExternalOutput").ap()

    with tile.TileContext(nc) as tc:
        with (
            tc.tile_pool(name="const", bufs=1) as cpool,
            tc.tile_pool(name="gp", bufs=4) as gpool,
            tc.tile_pool(name="zbp", bufs=4) as zbpool,
            tc.tile_pool(name="mt", bufs=8) as mtpool,
            tc.tile_pool(name="ep", bufs=3) as epool,
            tc.tile_pool(name="pacc", bufs=2, space="PSUM") as pacc,
            tc.tile_pool(name="ptr", bufs=2, space="PSUM") as ptr,
            tc.tile_pool(name="pout", bufs=2, space="PSUM") as pout,
        ):
            W_sb = cpool.tile([D, OUT], F32R)
            dW_sb = cpool.tile([D, OUT], F32R)
            nc.sync.dma_start(W_sb[:], W_d[:])
            nc.sync.dma_start(dW_sb[:], dW_d[:])
            iota_i = cpool.tile([P, P], I32)
            nc.gpsimd.iota(iota_i[:], pattern=[[1, P]], base=0,
                           channel_multiplier=0)
            iota_f = cpool.tile([P, P], F16)
            nc.vector.tensor_copy(iota_f[:], iota_i[:])
            ident = cpool.tile([P, P], FP)
            make_identity(nc, ident[:])
            idx_all = cpool.tile([P, n_tiles * cols_t], I16)
            nc.sync.dma_start(idx_all[:], idx_d[:, : n_tiles * cols_t])
            wd_all = cpool.tile([P, n_tiles * 2 * CH_T], FP)
            nc.gpsimd.dma_start(wd_all[:], wd_d[:, : n_tiles * 2 * CH_T])

            for t_iter in range(n_tiles * passes):
                t = t_iter % n_tiles
                zb_t = zbpool.tile([P, OUT], F16, tag="zb")
                nc.sync.dma_start(
                    zb_t[:NODES_PER_TILE, :],
                    zb_d[t * NODES_PER_TILE : (t + 1) * NODES_PER_TILE, :])

                G = gpool.tile([P, CH_T * 256], F16, tag="g")
                c0 = t * cols_t
                nc.gpsimd.dma_gather(
                    out_ap=G[:, : C_LO * 256].rearrange(
                        "p (c f) -> p c f", c=C_LO),
                    in_ap=table_d[:SPLIT, :],
                    idxs_ap=idx_all[:, c0 : c0 + C_LO * 8],
                    num_idxs=C_LO * P,
                    num_idxs_reg=C_LO * P,
                    elem_size=256,
                    single_packet=False,
                )
                nc.gpsimd.dma_gather(
                    out_ap=G[:, C_LO * 256 :].rearrange(
                        "p (c f) -> p c f", c=C_HI),
                    in_ap=table_d[SPLIT:, :],
                    idxs_ap=idx_all[:, c0 + C_LO * 8 : c0 + cols_t],
                    num_idxs=C_HI * P,
                    num_idxs_reg=C_HI * P,
                    elem_size=256,
                    single_packet=False,
                )

                # acc[dloc, 0:128]=AH  [128:256]=AdH  [256:384]=dAH  [384:512]=dAdH
                acc = pacc.tile([P, 512], FP, tag="acc")
                w0 = t * 2 * CH_T
                for ck in range(CH_T):
                    MT = mtpool.tile([P, P], F16, tag="mt")
                    nc.vector.tensor_scalar(
                        out=MT[:],
                        in0=iota_f[:],
                        scalar1=wd_all[:, w0 + 2 * ck : w0 + 2 * ck + 1],
                        scalar2=wd_all[:, w0 + 2 * ck + 1 : w0 + 2 * ck + 2],
                        op0=mybir.AluOpType.is_equal,
                        op1=mybir.AluOpType.mult,
                    )
                    dadj = (C_la <= ck < C_LO) or (ck >= C_LO + C_ha)
                    base = 256 if dadj else 0
                    nc.tensor.matmul(
                        acc[:, base : base + 256],
                        lhsT=MT[:],
                        rhs=G[:, ck * 256 : (ck + 1) * 256],
                        start=(ck == 0),
                        stop=(ck == CH_T - 1),
                    )

                # F = AdH + dAH + dAdH ; B = AH + F
                F_sb = epool.tile([P, P], FP, tag="ft")
                nc.vector.tensor_copy(F_sb[:], acc[:, 128:256])
                nc.vector.tensor_tensor(
                    out=F_sb[:], in0=F_sb[:], in1=acc[:, 256:384],
                    op=mybir.AluOpType.add)
                nc.vector.tensor_tensor(
                    out=F_sb[:], in0=F_sb[:], in1=acc[:, 384:512],
                    op=mybir.AluOpType.add)
                stage = epool.tile([P, 3 * OUT], F16, tag="stage")
                B_sb = epool.tile([P, P], FP, tag="bsb")
                nc.vector.tensor_tensor(
                    out=B_sb[:], in0=F_sb[:], in1=acc[:, 0:128],
                    op=mybir.AluOpType.add)
                nc.scalar.copy(stage[:, 256:384], B_sb[:])

                ft_ps = ptr.tile([P, P], FP, tag="ftp", space="PSUM")
                nc.tensor.transpose(ft_ps[:], F_sb[:], ident[:])
                bt_ps = ptr.tile([P, P], FP, tag="btp", space="PSUM")
                nc.tensor.transpose(bt_ps[:], B_sb[:], ident[:])
                FT_sb = epool.tile([P, P], F32R, tag="ftt")
                nc.scalar.copy(FT_sb[:], ft_ps[:])
                BT_sb = epool.tile([P, P], F32R, tag="btt")
                nc.scalar.copy(BT_sb[:], bt_ps[:])

                pz = pout.tile([P, 2 * OUT], FP, tag="pz", space="PSUM")
                nc.tensor.matmul(pz[:, 128:256], lhsT=FT_sb[:], rhs=W_sb[:],
                                 start=True, stop=False)
                nc.tensor.matmul(pz[:, 0:128], lhsT=FT_sb[:], rhs=W_sb[:],
                                 start=False, stop=False)
                nc.tensor.matmul(pz[:, 0:128], lhsT=BT_sb[:], rhs=dW_sb[:],
                                 start=False, stop=True)

                nc.scalar.copy(stage[:, 128:256], pz[:, 128:256])
                nc.vector.tensor_tensor(
                    out=stage[:NODES_PER_TILE, 0:128],
                    in0=pz[:NODES_PER_TILE, 0:128],
                    in1=zb_t[:NODES_PER_TILE, :],
                    op=mybir.AluOpType.add)

                r0 = t * NODES_PER_TILE
                nc.scalar.dma_start(
                    res_d[r0 : r0 + NODES_PER_TILE, :],
                    stage[:NODES_PER_TILE, :])

    nc.compile()
    _PROGRAM_CACHE[key] = nc
    return nc


# ----------------------------------------------------------------------------
# entry point
# ----------------------------------------------------------------------------

def _make_in_maps(inputs, meta, idx_cores, wd_cores, node_at):
    features = np.asarray(inputs["features"], dtype=np.float32)
    delta_features = np.asarray(inputs["delta_features"], dtype=np.float32)
    cached_Z = np.asarray(inputs["cached_Z"], dtype=np.float32)
    W = np.ascontiguousarray(np.asarray(inputs["W"], dtype=np.float32))
    dW = np.ascontiguousarray(np.asarray(inputs["delta_W"], dtype=np.float32))
    db = np.asarray(inputs["delta_bias"], dtype=np.float32)

    table = np.ascontiguousarray(
        np.concatenate([features, delta_features], axis=1).astype(np.float16))
    zb_perm = (cached_Z + db[None, :])[node_at].astype(np.float16)
    zb_cores = np.ascontiguousarray(
        zb_perm.reshape(NCORES, ROWS_PER_CORE, OUT))

    in_maps = []
    for c in range(NCORES):
        in_maps.append({
            "table": table,
            "idxs": np.ascontiguousarray(idx_cores[c]),
            "wd": np.ascontiguousarray(wd_cores[c]),
            "zb": zb_cores[c],
            "Wm": W,
            "dWm": dW,
        })
    return in_maps


def _assemble(results, node_at):
    res_rows = np.concatenate([r["res"] for r in results], axis=0)  # [N, 384]
    new_Z = np.empty((N, OUT), dtype=np.float32)
    fixed = np.empty((N, OUT), dtype=np.float32)
    B = np.empty((N, OUT), dtype=np.float32)
    new_Z[node_at] = res_rows[:, 0:128]
    fixed[node_at] = res_rows[:, 128:256]
    B[node_at] = res_rows[:, 256:384]
    return new_Z, fixed, B


def kernel(**inputs):
    meta, idx_cores, wd_cores, node_at, _, _ = _prep(inputs)
    nc = _build_program(meta)
    in_maps = _make_in_maps(inputs, meta, idx_cores, wd_cores, node_at)
    res = run_bass_kernel_spmd(nc, in_maps, list(range(NCORES)), trace=False)
    return _assemble(res.results, node_at)
